# revision 46
# baseline (speedup 1.0000x reference)
"""Trainium2 Bass kernel for nn_Attend_584115552611.

Attention B=4, H=16, N=2048, D=64 fp32 with the "swap" quirk:
when swap is truthy, attn probs of batches 0,1 are reused for batches 2,3
(each batch keeps its own v).  We therefore compute one softmax(QK^T) per
(qk-batch, head) "pair-unit" and apply it to two v tensors at once by
packing [v_b | v_{b+2}] into the PE stationary columns.

Sharding: 32 pair-units (2 qk-batches x 16 heads) spread over 8 cores,
4 units per core (data/head parallel, no collectives).

v2 pipeline (vs the ones-matmul baseline):
  * softmax denominators come for free from a 129th all-ones column in
    the AV matmul rhs (PSUM [q, 0:128]=out pair, [q,128]=sum) -- the
    separate ones-stationary sum matmuls (1/3 of PE work) are gone.
  * exp is split across BOTH the scalar engine (true exp) and the vector
    engine (custom 8-stage DVE op computing ((u+1)^2+1)^32 ~= 2^32 e^{32u}).
    The host pre-scales q by 2^-8 (exact) so PSUM scores arrive as
    u = s/256; ACT matches the 2^32 factor via bias=32*ln2, and the
    global 2^32 cancels in the normalization.
"""

import sys
import functools
import math

import numpy as np

for _p in ("/opt/trn_rl_repo",):
    if _p not in sys.path:
        sys.path.insert(0, _p)

import bass_rust
import concourse.bass as bass
import concourse.tile as tile
from concourse import mybir

B, H, N, D = 4, 16, 2048, 64
N_CORES = 8
FP32 = mybir.dt.float32
BF16 = mybir.dt.bfloat16
FP32R = mybir.dt.float32r
LN2_32 = 32.0 * math.log(2.0)


# ---------------------------------------------------------------------------
# Custom DVE op: exp32(u) = ((u+c0)^2 + c1)^32 ~= 2^32 * exp(32*u)
# (8 ALU stages exactly: ADD, SQ, ADD, SQ x5.)  c0=1.004, c1=2-c0^2 is the
# minimax skew: the base equals 2*(1 + a1*u + u^2/2) with a1=1.004, whose
# cubic log-error  (a1-1)s - s^3/6144  is Chebyshev-balanced on |s|<=5.8
# (max 0.86% vs 3.6% for the plain Taylor base).  base(0)=2 keeps the 2^32
# scale matched to the ACT half's bias=32*ln2.  Registered at import into
# concourse's custom-DVE table machinery with a self-computed uops sha.
# ---------------------------------------------------------------------------
EXP_C0 = 1.004
EXP_C1 = 2.0 - EXP_C0 * EXP_C0


@functools.lru_cache(maxsize=1)
def _get_exp32_op():
    from concourse import dve_ops as dops
    from concourse.dve_spec import Spec, Src0, C0, C1, sq, lower, _has_src1
    from concourse.dve_uop import DveOpSpec

    name = "EXP32_ATTEND_ANT"
    if name in dops._SUB_OPCODE_FOR_NAME:
        for op in dops.OPS:
            if op.name == name:
                return op

    a = Src0 + C0
    c = sq(a) + C1
    body = sq(sq(sq(sq(sq(c)))))

    def _ref(in0, in1, s0, s1, imm2):
        x = (in0.astype(np.float32) + np.float32(s0)) ** 2 + np.float32(s1)
        for _ in range(5):
            x = x * x
        return x

    spec = Spec(body=body, reference=_ref)
    row = max(dops._SUB_OPCODE_FOR_NAME.values()) + 1
    assert row < 0x20, "no free custom-DVE opcode rows"
    dops._SUB_OPCODE_FOR_NAME[name] = row
    shas = {}
    for ver in ("v3", "v4"):
        s = DveOpSpec(
            name=name, opcode=row, uops=lower(spec, ver=ver), rd1_en=_has_src1(spec)
        )
        shas[ver] = s.sha(ver)
    op = dops.DveOp(name, spec, subdim=False, uops_sha=shas)
    dops.OPS.append(op)
    return op


def _split_excess_waits(nc, maxw=1):
    """This walrus build rejects instructions carrying more than one sync
    wait: spread excess waits onto inserted same-engine NOPs just before
    the offending instruction (engine queues are in-order, so semantics
    are unchanged)."""
    nid = 0
    for f in nc.m.functions:
        for bb in f.blocks:
            out = []
            changed = False
            for inst in bb.instructions:
                si = inst.sync_info
                waits = list(si.on_wait) if si and si.on_wait else []
                if len(waits) > maxw:
                    changed = True
                    for w in waits[:-maxw]:
                        nid += 1
                        nop = mybir.InstNoOp(name=f"I-waitsplit-{nid}")
                        nop.engine = inst.engine
                        nop.sync_info = bass_rust.SyncInfo(on_wait=[w], on_update=[])
                        out.append(nop)
                    si.on_wait = waits[-maxw:]
                out.append(inst)
            if changed:
                bb.instructions = out


import os

QK_BUFS = int(os.environ.get("ATT_QK_BUFS", "3"))
AV_BUFS = int(os.environ.get("ATT_AV_BUFS", "2"))
PIPE_LAG = int(os.environ.get("ATT_LAG", "2"))
ACT_J = int(os.environ.get("ATT_ACT_J", "9"))  # ACT exp share in 16ths
NORM_ACT = int(os.environ.get("ATT_NORM_ACT", "1"))  # norms on ACT out of 4
WARM_MM = int(os.environ.get("ATT_WARM", "5"))
INS_BUFS = int(os.environ.get("ATT_INS_BUFS", "3"))
PT_BUFS = int(os.environ.get("ATT_PT_BUFS", "3"))
AV_W = int(os.environ.get("ATT_AV_W", "512"))
# 2-byte q/k mode: halves the startup DMA prefix at 1.0 cyc/row (same as
# fp32r).  fp16 (10-bit mantissa) adds only ~0.07% score noise vs bf16's
# 0.28%; the prescale splits as q*2^-4, k*2^-4 so values sit in fp16's
# normal range and PSUM still receives u = s_raw/256.
QK_F16 = int(os.environ.get("ATT_QK_F16", "1"))
QK_DT = mybir.dt.float16 if QK_F16 else FP32R


def build_attn_program(n_units, n_ctx=N, d=D):
    """One softmax(q k^T * d^-0.5) per unit applied to TWO v tensors.

    Host-packed operand layouts (per unit):
      qt  [128, T*128]    f16  rows 0:64 = q^T * 2^-4 (d-major), rows 64:128
                               = copy (replica feeds PE row-group 1)
      kt  [128, T/2*128]  f16  rows 0:64 = k^T * 2^-4 of even 128-row
                               k-tiles, rows 64:128 = odd (row-group packing)
      vv  [128, T*129]    bf16 [p, t*129 + w*64+dd] = v_w[t*128+p, dd],
                               col 128 of each t-block = 1.0 (sum column)
      out [n_ctx, 128]    f32  [q, w*64+dd] -- plain q-major, no transpose
    """
    assert d == 64 and n_ctx % 512 == 0
    T = n_ctx // 128          # k/q tiles of 128 rows
    NCH = n_ctx // 512        # 512-wide q chunks
    exp32 = _get_exp32_op()

    nc = bass.Bass()
    qt = nc.declare_dram_parameter("qt", [n_units, 128, T * 128], QK_DT, isOutput=False)
    kt = nc.declare_dram_parameter(
        "kt", [n_units, 128, (T // 2) * 128], QK_DT, isOutput=False
    )
    vv = nc.declare_dram_parameter("vv", [n_units, 128, T * 129], BF16, isOutput=False)
    out = nc.declare_dram_parameter("out", [n_units, n_ctx, 128], FP32, isOutput=True)

    with tile.TileContext(nc) as tc:
        with (
            tc.tile_pool(name="singles", bufs=1) as singles,
            tc.tile_pool(name="ins", bufs=INS_BUFS) as ins_pool,
            tc.tile_pool(name="pt", bufs=PT_BUFS) as pt_pool,
            tc.tile_pool(name="sig", bufs=8) as sig_pool,
            tc.tile_pool(name="outs", bufs=8) as outs_pool,
            tc.tile_pool(name="qk_ps", bufs=QK_BUFS, space="PSUM") as qk_ps_pool,
            tc.tile_pool(name="av_ps", bufs=AV_BUFS, space="PSUM") as av_ps_pool,
        ):
            bias_t = singles.tile([128, 1], FP32)
            nc.vector.memset(bias_t, LN2_32)
            warm_rhs = singles.tile([128, 512], BF16)
            nc.vector.memset(warm_rhs, 1.0)
            # Dummy exp pulls the ~1.3us ACT_TABLE_LOAD for the exp table
            # set into the idle startup window; otherwise the first real
            # exp pays it in the middle of the pipeline-fill phase.
            act_scr = singles.tile([128, 1], FP32)
            nc.scalar.activation(
                out=act_scr,
                in_=bias_t,
                func=mybir.ActivationFunctionType.Exp,
                scale=0.0,
            )

            # Warm up the PE (HAM clock gate) while the first unit's DMA
            # loads are in flight: ~3.3us of dummy matmuls so real work
            # starts at 2.4 GHz instead of the cold 1.2 GHz.  The warm psum
            # is discarded (later accumulation chains start=True reset it).
            warm = av_ps_pool.tile([128, 512], FP32, tag="av")
            for _ in range(WARM_MM):
                nc.tensor.matmul(
                    warm, lhsT=warm_rhs[:, 0:128], rhs=warm_rhs, start=True, stop=True
                )

            # ---- flat software pipeline over all (unit, chunk) slots:
            # per iteration i: consume slot i-LAG (AV + normalize + store),
            # then produce slot i (loads, QK + exp), ACROSS unit boundaries.
            # LAG=2 so the exp of slot i-1 (on ACT+DVE) fully overlaps the
            # PE's AV(i-2) + QK(i) work instead of serializing with it.
            LAG = PIPE_LAG
            slots = [(u, c) for u in range(n_units) for c in range(NCH)]
            ins_tiles = {}
            pT_tiles = {}

            def consume_qt(u2, c2, pT, vv2, s4, norm_act=None):
                """AV + sums for one q-tile: stationary pT q-tile, rhs
                [v0|v1|ones]; then normalize + store."""
                qt_i = c2 * 4 + s4
                av = av_ps_pool.tile([128, AV_W], FP32, tag="av")
                for t in range(T):
                    nc.tensor.matmul(
                        av[:, 0:129],
                        lhsT=pT[:, t, s4 * 128 : (s4 + 1) * 128],
                        rhs=vv2[:, t, :],
                        start=(t == 0),
                        stop=(t == T - 1),
                    )
                rec = sig_pool.tile([128, 1], FP32, tag="rec")
                nc.vector.reciprocal(out=rec, in_=av[:, 128:129])
                o_sb = outs_pool.tile([128, 128], FP32, tag="o_sb")
                # out = av * (1/sum): per-partition scale, split between
                # ACT and DVE per NORM_ACT (engine balance)
                if not (norm_act if norm_act is not None else s4 < NORM_ACT):
                    nc.vector.tensor_scalar_mul(o_sb, av[:, 0:128], rec)
                else:
                    nc.scalar.activation(
                        out=o_sb,
                        in_=av[:, 0:128],
                        func=mybir.ActivationFunctionType.Copy,
                        scale=rec,
                    )
                nc.sync.dma_start(
                    out=out[u2, qt_i * 128 : (qt_i + 1) * 128, :], in_=o_sb
                )

            def produce_j(u, c, pT, qT_rep, kT_st, j):
                """QK for one j (two k-tiles via PE row groups) + exp."""
                qs = c * 4
                ps = qk_ps_pool.tile([128, 1024], FP32, tag="qk")
                # row-group 0: k-tile 2j ; row-group 1: k-tile 2j+1
                nc.tensor.matmul(
                    ps[:, 0:512],
                    lhsT=kT_st[0:64, j, :],
                    rhs=qT_rep[0:64, qs : qs + 4, :],
                    start=True,
                    stop=True,
                )
                nc.tensor.matmul(
                    ps[:, 512:1024],
                    lhsT=kT_st[64:128, j, :],
                    rhs=qT_rep[64:128, qs : qs + 4, :],
                    start=True,
                    stop=True,
                )
                # unnormalized probs (x 2^32), split across both engines:
                # psum already holds u = s/256.  ACT takes ACT_J of every
                # 16 js (across chunk pairs).
                on_act = ((c * (T // 2) + j) * ACT_J) % 16 < ACT_J
                if on_act:
                    nc.scalar.activation(
                        out=pT[:, 2 * j : 2 * j + 2, :],
                        in_=ps,
                        func=mybir.ActivationFunctionType.Exp,
                        scale=32.0,
                        bias=bias_t,
                    )
                else:
                    nc.vector._custom_dve(
                        exp32,
                        out=pT[:, 2 * j : 2 * j + 2, :],
                        in0=ps[:, :],
                        s0=EXP_C0,
                        s1=EXP_C1,
                    )

            for i in range(len(slots) + LAG):
                # consume slot i-LAG, produce slot i, INTERLEAVED in halves
                # so slot i's exps start mid-iteration (deeper exp overlap).
                cons = prod = None
                if i >= LAG:
                    u2, c2 = slots[i - LAG]
                    _, _, vv2 = ins_tiles[u2]
                    pT2 = pT_tiles.pop((u2, c2))
                    cons = (u2, c2, pT2, vv2)
                if i < len(slots):
                    u, c = slots[i]
                    if c == 0:
                        # loads (contiguous; HWDGE).  kt + first q chunk
                        # first: the first QK needs only those.
                        qT_rep = ins_pool.tile([128, T, 128], QK_DT, tag="qT")
                        kT_st = ins_pool.tile([128, T // 2, 128], QK_DT, tag="kT")
                        vv_sb = ins_pool.tile([128, T, 129], BF16, tag="vv_sb")
                        qt_r = qt[u].rearrange("p (t r) -> p t r", t=T)
                        nc.sync.dma_start(
                            out=kT_st,
                            in_=kt[u].rearrange("p (j r) -> p j r", j=T // 2),
                        )
                        for qc in range(NCH):
                            nc.sync.dma_start(
                                out=qT_rep[:, 4 * qc : 4 * qc + 4, :],
                                in_=qt_r[:, 4 * qc : 4 * qc + 4, :],
                            )
                        nc.sync.dma_start(
                            out=vv_sb, in_=vv[u].rearrange("p (t r) -> p t r", t=T)
                        )
                        ins_tiles[u] = (qT_rep, kT_st, vv_sb)
                    qT_rep, kT_st, vv_sb = ins_tiles[u]
                    pT = pt_pool.tile([128, T, 512], BF16, tag="pT")
                    pT_tiles[u, c] = pT
                    prod = (u, c, pT, qT_rep, kT_st)

                if cons is not None:
                    consume_qt(*cons, 0)
                    consume_qt(*cons, 1)
                if prod is not None:
                    for j in range(0, T // 4):
                        produce_j(*prod, j)
                if cons is not None:
                    consume_qt(*cons, 2)
                    consume_qt(*cons, 3)
                    if cons[1] == NCH - 1:
                        ins_tiles.pop(cons[0])
                if prod is not None:
                    for j in range(T // 4, T // 2):
                        produce_j(*prod, j)

    _split_excess_waits(nc)
    # Encode InstISA subclass bytes (fills the custom-DVE instruction's
    # `instr` payload; without this walrus rejects it with "ISA wrong length")
    mybir.codegen_inst_isa_subclasses(nc)
    return nc


@functools.lru_cache(maxsize=4)
def _get_program(n_units, n_ctx):
    return build_attn_program(n_units, n_ctx)


def _get_runner(n_units, n_ctx):
    """Build the bass program once and return a cached jitted SPMD runner:
    runner(concat_qk, concat_vv) -> concat_out  (axis 0 = cores*units)."""
    import jax
    from jax.experimental.shard_map import shard_map
    from jax.sharding import Mesh, PartitionSpec
    from concourse import bass2jax

    try:
        jax.config.update("jax_compilation_cache_dir", "/tmp/jax_neff_cache")
        jax.config.update("jax_persistent_cache_min_compile_time_secs", 10)
    except Exception:
        pass
    bass2jax.install_neuronx_cc_hook()
    nc = _get_program(n_units, n_ctx)

    in_names, out_names, out_avals, zero_shapes = [], [], [], []
    for alloc in nc.m.functions[0].allocations:
        if not isinstance(alloc, mybir.MemoryLocationSet):
            continue
        name = alloc.memorylocations[0].name
        if alloc.kind == "ExternalInput":
            if nc.partition_id_tensor is None or name != nc.partition_id_tensor.name:
                in_names.append(name)
        elif alloc.kind == "ExternalOutput":
            out_names.append(name)
            shape = tuple(alloc.tensor_shape)
            dtype = mybir.dt.np(alloc.dtype)
            out_avals.append(jax.core.ShapedArray(shape, dtype))
            zero_shapes.append((shape, dtype))
    assert in_names == ["qt", "kt", "vv"] and out_names == ["out"]
    n_params = len(in_names)
    all_names = in_names + out_names
    if nc.partition_id_tensor is not None:
        all_names.append(nc.partition_id_tensor.name)

    def _body(*args):
        operands = list(args)
        if nc.partition_id_tensor is not None:
            operands.append(bass2jax.partition_id_tensor())
        outs = bass2jax._bass_exec_p.bind(
            *operands,
            out_avals=tuple(out_avals),
            in_names=tuple(all_names),
            out_names=tuple(out_names),
            lowering_input_output_aliases=(),
            sim_require_finite=True,
            sim_require_nnan=True,
            nc=nc,
        )
        return tuple(outs)

    devices = jax.devices()[:N_CORES]
    mesh = Mesh(np.asarray(devices), ("core",))
    n_outs = len(out_names)
    sharded = jax.jit(
        shard_map(
            _body,
            mesh=mesh,
            in_specs=(PartitionSpec("core"),) * (n_params + n_outs),
            out_specs=(PartitionSpec("core"),) * n_outs,
            check_rep=False,
        ),
        keep_unused=True,
    )

    def runner(qt_all, kt_all, vv_all):
        zeros = [
            np.zeros((N_CORES * s[0], *s[1:]), dt) for (s, dt) in zero_shapes
        ]
        (out_all,) = sharded(qt_all, kt_all, vv_all, *zeros)
        return np.asarray(out_all)

    runner.sharded = sharded
    runner.mesh = mesh
    runner.zero_shapes = zero_shapes
    return runner


_RUNNERS = {}


def _pack_inputs(unit_specs, q, k, v, n_ctx):
    """unit_specs: list of (qk_batch, head, v_batch0, v_batch1)."""
    import ml_dtypes

    T = n_ctx // 128
    NU = len(unit_specs)
    qk_np = np.float16 if QK_F16 else np.float32
    q_pre = np.float32(2.0**-4 if QK_F16 else 2.0**-8)
    k_pre = np.float32(2.0**-4 if QK_F16 else 1.0)
    qt_all = np.empty((NU, 128, T * 128), qk_np)
    kt_all = np.empty((NU, 128, (T // 2) * 128), qk_np)
    vv_all = np.empty((NU, 128, T * 129), ml_dtypes.bfloat16)
    vtmp = np.empty((T, 128, 129), np.float32)
    vtmp[:, :, 128] = 1.0
    for i, (bq, h, b0, b1) in enumerate(unit_specs):
        qT = q[bq, h].T * q_pre              # [64, n_ctx], d-major, prescaled
        qt_all[i, 0:64] = qT
        qt_all[i, 64:128] = qT               # replica feeds PE row-group 1
        kT = (k[bq, h].T * k_pre).reshape(64, T, 128)  # [dd, t, r]
        kt_all[i, 0:64] = kT[:, 0::2, :].reshape(64, -1)
        kt_all[i, 64:128] = kT[:, 1::2, :].reshape(64, -1)
        vtmp[:, :, 0:64] = v[b0, h].reshape(T, 128, D)
        vtmp[:, :, 64:128] = v[b1, h].reshape(T, 128, D)
        vv_all[i] = (
            vtmp.transpose(1, 0, 2).reshape(128, -1).astype(ml_dtypes.bfloat16)
        )
    return qt_all, kt_all, vv_all


def _run_units(unit_specs, q, k, v, n_ctx):
    n_units = len(unit_specs) // N_CORES
    assert n_units * N_CORES == len(unit_specs)
    key = (n_units, n_ctx)
    if key not in _RUNNERS:
        _RUNNERS[key] = _get_runner(n_units, n_ctx)
    runner = _RUNNERS[key]

    qt_all, kt_all, vv_all = _pack_inputs(unit_specs, q, k, v, n_ctx)
    out_all = runner(qt_all, kt_all, vv_all)  # [NU, n_ctx, 128]

    out = np.empty((B, H, n_ctx, D), np.float32)
    for i, (bq, h, b0, b1) in enumerate(unit_specs):
        out[b0, h] = out_all[i, :, 0:64]
        if b1 != b0:
            out[b1, h] = out_all[i, :, 64:128]
    return out


def kernel(q, k, v, swap):
    q = np.ascontiguousarray(np.asarray(q, dtype=np.float32))
    k = np.ascontiguousarray(np.asarray(k, dtype=np.float32))
    v = np.ascontiguousarray(np.asarray(v, dtype=np.float32))
    swap_val = int(np.asarray(swap).reshape(-1)[0])

    n_ctx = q.shape[2]
    if swap_val:
        # 32 pair-units: attn of (b, h) applied to v[b] and v[b + B//2]
        specs = [(bq, h, bq, bq + B // 2) for bq in range(B // 2) for h in range(H)]
    else:
        # 64 independent units (2nd v slot duplicates the 1st)
        specs = [(b, h, b, b) for b in range(B) for h in range(H)]
    return _run_units(specs, q, k, v, n_ctx)


if __name__ == "__main__":
    rng = np.random.default_rng(0)
    q = rng.standard_normal((B, H, N, D), dtype=np.float32)
    k = rng.standard_normal((B, H, N, D), dtype=np.float32)
    v = rng.standard_normal((B, H, N, D), dtype=np.float32)
    o = kernel(q, k, v, 1)
    print("out", o.shape, o.dtype, float(np.abs(o).mean()))


# revision 50
# speedup vs baseline: 1.0181x; 1.0181x over previous
"""Trainium2 Bass kernel for nn_Attend_584115552611.

Attention B=4, H=16, N=2048, D=64 fp32 with the "swap" quirk:
when swap is truthy, attn probs of batches 0,1 are reused for batches 2,3
(each batch keeps its own v).  We therefore compute one softmax(QK^T) per
(qk-batch, head) "pair-unit" and apply it to two v tensors at once by
packing [v_b | v_{b+2}] into the PE stationary columns.

Sharding: 32 pair-units (2 qk-batches x 16 heads) spread over 8 cores,
4 units per core (data/head parallel, no collectives).

Pipeline (vs the ones-matmul baseline):
  * softmax denominators come for free from a 129th all-ones column in
    the AV matmul rhs (PSUM [q, 0:128]=out pair, [q,128]=sum) -- the
    separate ones-stationary sum matmuls (1/3 of PE work) are gone.
  * exp is split across BOTH the scalar engine (true exp) and the vector
    engine (custom 8-stage DVE op computing ((u+c0)^2+c1)^32 ~= 2^32
    e^{32u}).  The host pre-scales q and k by 2^-4 each (exact) so PSUM
    scores arrive as u = s_raw/256; ACT matches the 2^32 factor via
    bias=32*ln2, and the global 2^32 cancels in the normalization.
  * q/k ship as fp16 (halves the startup DMA prefix, ~0.07% score noise),
    the ACT exp table is preloaded during the startup window, and the
    per-iteration schedule interleaves [AV qt0-1 | QK j0-3 | AV qt2-3 |
    QK j4-7] at consume-lag 2 for 92%+ PE occupancy.
"""

import sys
import functools
import math

import numpy as np

for _p in ("/opt/trn_rl_repo",):
    if _p not in sys.path:
        sys.path.insert(0, _p)

import bass_rust
import concourse.bass as bass
import concourse.tile as tile
from concourse import mybir

B, H, N, D = 4, 16, 2048, 64
N_CORES = 8
FP32 = mybir.dt.float32
BF16 = mybir.dt.bfloat16
FP32R = mybir.dt.float32r
LN2_32 = 32.0 * math.log(2.0)


# ---------------------------------------------------------------------------
# Custom DVE op: exp32(u) = ((u+c0)^2 + c1)^32 ~= 2^32 * exp(32*u)
# (8 ALU stages exactly: ADD, SQ, ADD, SQ x5.)  c0=1.004, c1=2-c0^2 is the
# minimax skew: the base equals 2*(1 + a1*u + u^2/2) with a1=1.004, whose
# cubic log-error  (a1-1)s - s^3/6144  is Chebyshev-balanced on |s|<=5.8
# (max 0.86% vs 3.6% for the plain Taylor base).  base(0)=2 keeps the 2^32
# scale matched to the ACT half's bias=32*ln2.  Registered at import into
# concourse's custom-DVE table machinery with a self-computed uops sha.
# ---------------------------------------------------------------------------
EXP_C0 = 1.004
EXP_C1 = 2.0 - EXP_C0 * EXP_C0


@functools.lru_cache(maxsize=1)
def _get_exp32_op():
    from concourse import dve_ops as dops
    from concourse.dve_spec import Spec, Src0, C0, C1, sq, lower, _has_src1
    from concourse.dve_uop import DveOpSpec

    name = "EXP32_ATTEND_ANT"
    if name in dops._SUB_OPCODE_FOR_NAME:
        for op in dops.OPS:
            if op.name == name:
                return op

    a = Src0 + C0
    c = sq(a) + C1
    body = sq(sq(sq(sq(sq(c)))))

    def _ref(in0, in1, s0, s1, imm2):
        x = (in0.astype(np.float32) + np.float32(s0)) ** 2 + np.float32(s1)
        for _ in range(5):
            x = x * x
        return x

    spec = Spec(body=body, reference=_ref)
    row = max(dops._SUB_OPCODE_FOR_NAME.values()) + 1
    assert row < 0x20, "no free custom-DVE opcode rows"
    dops._SUB_OPCODE_FOR_NAME[name] = row
    shas = {}
    for ver in ("v3", "v4"):
        s = DveOpSpec(
            name=name, opcode=row, uops=lower(spec, ver=ver), rd1_en=_has_src1(spec)
        )
        shas[ver] = s.sha(ver)
    op = dops.DveOp(name, spec, subdim=False, uops_sha=shas)
    dops.OPS.append(op)
    return op


def _split_excess_waits(nc, maxw=1):
    """This walrus build rejects instructions carrying more than one sync
    wait: spread excess waits onto inserted same-engine NOPs just before
    the offending instruction (engine queues are in-order, so semantics
    are unchanged)."""
    nid = 0
    for f in nc.m.functions:
        for bb in f.blocks:
            out = []
            changed = False
            for inst in bb.instructions:
                si = inst.sync_info
                waits = list(si.on_wait) if si and si.on_wait else []
                if len(waits) > maxw:
                    changed = True
                    for w in waits[:-maxw]:
                        nid += 1
                        nop = mybir.InstNoOp(name=f"I-waitsplit-{nid}")
                        nop.engine = inst.engine
                        nop.sync_info = bass_rust.SyncInfo(on_wait=[w], on_update=[])
                        out.append(nop)
                    si.on_wait = waits[-maxw:]
                out.append(inst)
            if changed:
                bb.instructions = out


import os

QK_BUFS = int(os.environ.get("ATT_QK_BUFS", "3"))
AV_BUFS = int(os.environ.get("ATT_AV_BUFS", "2"))
PIPE_LAG = int(os.environ.get("ATT_LAG", "2"))
ACT_J = int(os.environ.get("ATT_ACT_J", "9"))  # ACT exp share in 16ths
NORM_ACT = int(os.environ.get("ATT_NORM_ACT", "1"))  # norms on ACT out of 4
WARM_MM = int(os.environ.get("ATT_WARM", "5"))
INS_BUFS = int(os.environ.get("ATT_INS_BUFS", "3"))
PT_BUFS = int(os.environ.get("ATT_PT_BUFS", "3"))
AV_W = int(os.environ.get("ATT_AV_W", "512"))
# 2-byte q/k mode: halves the startup DMA prefix at 1.0 cyc/row (same as
# fp32r).  fp16 (10-bit mantissa) adds only ~0.07% score noise vs bf16's
# 0.28%; the prescale splits as q*2^-4, k*2^-4 so values sit in fp16's
# normal range and PSUM still receives u = s_raw/256.
QK_F16 = int(os.environ.get("ATT_QK_F16", "1"))
QK_DT = mybir.dt.float16 if QK_F16 else FP32R


def build_attn_program(n_units, n_ctx=N, d=D):
    """One softmax(q k^T * d^-0.5) per unit applied to TWO v tensors.

    Host-packed operand layouts (per unit):
      qt  [128, T*128]    f16  rows 0:64 = q^T * 2^-4 (d-major), rows 64:128
                               = copy (replica feeds PE row-group 1)
      kt  [128, T/2*128]  f16  rows 0:64 = k^T * 2^-4 of even 128-row
                               k-tiles, rows 64:128 = odd (row-group packing)
      vv  [128, T*129]    bf16 [p, t*129 + w*64+dd] = v_w[t*128+p, dd],
                               col 128 of each t-block = 1.0 (sum column)
      out [n_ctx, 128]    f32  [q, w*64+dd] -- plain q-major, no transpose
    """
    assert d == 64 and n_ctx % 512 == 0
    T = n_ctx // 128          # k/q tiles of 128 rows
    NCH = n_ctx // 512        # 512-wide q chunks
    exp32 = _get_exp32_op()

    nc = bass.Bass()
    qt = nc.declare_dram_parameter("qt", [n_units, 128, T * 128], QK_DT, isOutput=False)
    kt = nc.declare_dram_parameter(
        "kt", [n_units, 128, (T // 2) * 128], QK_DT, isOutput=False
    )
    vv = nc.declare_dram_parameter("vv", [n_units, 128, T * 129], BF16, isOutput=False)
    out = nc.declare_dram_parameter("out", [n_units, n_ctx, 128], FP32, isOutput=True)

    with tile.TileContext(nc) as tc:
        with (
            tc.tile_pool(name="singles", bufs=1) as singles,
            tc.tile_pool(name="ins", bufs=INS_BUFS) as ins_pool,
            tc.tile_pool(name="pt", bufs=PT_BUFS) as pt_pool,
            tc.tile_pool(name="sig", bufs=8) as sig_pool,
            tc.tile_pool(name="outs", bufs=8) as outs_pool,
            tc.tile_pool(name="qk_ps", bufs=QK_BUFS, space="PSUM") as qk_ps_pool,
            tc.tile_pool(name="av_ps", bufs=AV_BUFS, space="PSUM") as av_ps_pool,
        ):
            bias_t = singles.tile([128, 1], FP32)
            nc.vector.memset(bias_t, LN2_32)
            warm_rhs = singles.tile([128, 512], BF16)
            nc.vector.memset(warm_rhs, 1.0)
            # Dummy exp pulls the ~1.3us ACT_TABLE_LOAD for the exp table
            # set into the idle startup window; otherwise the first real
            # exp pays it in the middle of the pipeline-fill phase.
            act_scr = singles.tile([128, 1], FP32)
            nc.scalar.activation(
                out=act_scr,
                in_=bias_t,
                func=mybir.ActivationFunctionType.Exp,
                scale=0.0,
            )

            # Warm up the PE (HAM clock gate) while the first unit's DMA
            # loads are in flight: ~3.3us of dummy matmuls so real work
            # starts at 2.4 GHz instead of the cold 1.2 GHz.  The warm psum
            # is discarded (later accumulation chains start=True reset it).
            warm = av_ps_pool.tile([128, 512], FP32, tag="av")
            for _ in range(WARM_MM):
                nc.tensor.matmul(
                    warm, lhsT=warm_rhs[:, 0:128], rhs=warm_rhs, start=True, stop=True
                )

            # ---- flat software pipeline over all (unit, chunk) slots:
            # per iteration i: consume slot i-LAG (AV + normalize + store),
            # then produce slot i (loads, QK + exp), ACROSS unit boundaries.
            # LAG=2 so the exp of slot i-1 (on ACT+DVE) fully overlaps the
            # PE's AV(i-2) + QK(i) work instead of serializing with it.
            LAG = PIPE_LAG
            slots = [(u, c) for u in range(n_units) for c in range(NCH)]
            ins_tiles = {}
            pT_tiles = {}

            def consume_qt(u2, c2, pT, vv2, s4, norm_act=None):
                """AV + sums for one q-tile: stationary pT q-tile, rhs
                [v0|v1|ones]; then normalize + store."""
                qt_i = c2 * 4 + s4
                av = av_ps_pool.tile([128, AV_W], FP32, tag="av")
                for t in range(T):
                    nc.tensor.matmul(
                        av[:, 0:129],
                        lhsT=pT[:, t, s4 * 128 : (s4 + 1) * 128],
                        rhs=vv2[:, t, :],
                        start=(t == 0),
                        stop=(t == T - 1),
                    )
                rec = sig_pool.tile([128, 1], FP32, tag="rec")
                nc.vector.reciprocal(out=rec, in_=av[:, 128:129])
                o_sb = outs_pool.tile([128, 128], FP32, tag="o_sb")
                # out = av * (1/sum): per-partition scale, split between
                # ACT and DVE per NORM_ACT (engine balance)
                if not (norm_act if norm_act is not None else s4 < NORM_ACT):
                    nc.vector.tensor_scalar_mul(o_sb, av[:, 0:128], rec)
                else:
                    nc.scalar.activation(
                        out=o_sb,
                        in_=av[:, 0:128],
                        func=mybir.ActivationFunctionType.Copy,
                        scale=rec,
                    )
                nc.sync.dma_start(
                    out=out[u2, qt_i * 128 : (qt_i + 1) * 128, :], in_=o_sb
                )

            def produce_j(u, c, pT, qT_rep, kT_st, j):
                """QK for one j (two k-tiles via PE row groups) + exp."""
                qs = c * 4
                ps = qk_ps_pool.tile([128, 1024], FP32, tag="qk")
                # row-group 0: k-tile 2j ; row-group 1: k-tile 2j+1
                nc.tensor.matmul(
                    ps[:, 0:512],
                    lhsT=kT_st[0:64, j, :],
                    rhs=qT_rep[0:64, qs : qs + 4, :],
                    start=True,
                    stop=True,
                )
                nc.tensor.matmul(
                    ps[:, 512:1024],
                    lhsT=kT_st[64:128, j, :],
                    rhs=qT_rep[64:128, qs : qs + 4, :],
                    start=True,
                    stop=True,
                )
                # unnormalized probs (x 2^32), split across both engines:
                # psum already holds u = s/256.  ACT takes ACT_J of every
                # 16 js (across chunk pairs).
                on_act = ((c * (T // 2) + j) * ACT_J) % 16 < ACT_J
                if on_act:
                    nc.scalar.activation(
                        out=pT[:, 2 * j : 2 * j + 2, :],
                        in_=ps,
                        func=mybir.ActivationFunctionType.Exp,
                        scale=32.0,
                        bias=bias_t,
                    )
                else:
                    nc.vector._custom_dve(
                        exp32,
                        out=pT[:, 2 * j : 2 * j + 2, :],
                        in0=ps[:, :],
                        s0=EXP_C0,
                        s1=EXP_C1,
                    )

            for i in range(len(slots) + LAG):
                # consume slot i-LAG, produce slot i, INTERLEAVED in halves
                # so slot i's exps start mid-iteration (deeper exp overlap).
                cons = prod = None
                if i >= LAG:
                    u2, c2 = slots[i - LAG]
                    _, _, vv2 = ins_tiles[u2]
                    pT2 = pT_tiles.pop((u2, c2))
                    cons = (u2, c2, pT2, vv2)
                if i < len(slots):
                    u, c = slots[i]
                    if c == 0:
                        # loads (contiguous; HWDGE).  kt + first q chunk
                        # first: the first QK needs only those.
                        qT_rep = ins_pool.tile([128, T, 128], QK_DT, tag="qT")
                        kT_st = ins_pool.tile([128, T // 2, 128], QK_DT, tag="kT")
                        vv_sb = ins_pool.tile([128, T, 129], BF16, tag="vv_sb")
                        qt_r = qt[u].rearrange("p (t r) -> p t r", t=T)
                        nc.sync.dma_start(
                            out=kT_st,
                            in_=kt[u].rearrange("p (j r) -> p j r", j=T // 2),
                        )
                        for qc in range(NCH):
                            nc.sync.dma_start(
                                out=qT_rep[:, 4 * qc : 4 * qc + 4, :],
                                in_=qt_r[:, 4 * qc : 4 * qc + 4, :],
                            )
                        nc.sync.dma_start(
                            out=vv_sb, in_=vv[u].rearrange("p (t r) -> p t r", t=T)
                        )
                        ins_tiles[u] = (qT_rep, kT_st, vv_sb)
                    qT_rep, kT_st, vv_sb = ins_tiles[u]
                    pT = pt_pool.tile([128, T, 512], BF16, tag="pT")
                    pT_tiles[u, c] = pT
                    prod = (u, c, pT, qT_rep, kT_st)

                for step in range(4):
                    if cons is not None:
                        consume_qt(*cons, step)
                    if prod is not None:
                        produce_j(*prod, 2 * step)
                        produce_j(*prod, 2 * step + 1)
                if cons is not None and cons[1] == NCH - 1:
                    ins_tiles.pop(cons[0])

    _split_excess_waits(nc)
    # Encode InstISA subclass bytes (fills the custom-DVE instruction's
    # `instr` payload; without this walrus rejects it with "ISA wrong length")
    mybir.codegen_inst_isa_subclasses(nc)
    return nc


@functools.lru_cache(maxsize=4)
def _get_program(n_units, n_ctx):
    return build_attn_program(n_units, n_ctx)


def _get_runner(n_units, n_ctx):
    """Build the bass program once and return a cached jitted SPMD runner:
    runner(concat_qk, concat_vv) -> concat_out  (axis 0 = cores*units)."""
    import jax
    from jax.experimental.shard_map import shard_map
    from jax.sharding import Mesh, PartitionSpec
    from concourse import bass2jax

    try:
        jax.config.update("jax_compilation_cache_dir", "/tmp/jax_neff_cache")
        jax.config.update("jax_persistent_cache_min_compile_time_secs", 10)
    except Exception:
        pass
    bass2jax.install_neuronx_cc_hook()
    nc = _get_program(n_units, n_ctx)

    in_names, out_names, out_avals, zero_shapes = [], [], [], []
    for alloc in nc.m.functions[0].allocations:
        if not isinstance(alloc, mybir.MemoryLocationSet):
            continue
        name = alloc.memorylocations[0].name
        if alloc.kind == "ExternalInput":
            if nc.partition_id_tensor is None or name != nc.partition_id_tensor.name:
                in_names.append(name)
        elif alloc.kind == "ExternalOutput":
            out_names.append(name)
            shape = tuple(alloc.tensor_shape)
            dtype = mybir.dt.np(alloc.dtype)
            out_avals.append(jax.core.ShapedArray(shape, dtype))
            zero_shapes.append((shape, dtype))
    assert in_names == ["qt", "kt", "vv"] and out_names == ["out"]
    n_params = len(in_names)
    all_names = in_names + out_names
    if nc.partition_id_tensor is not None:
        all_names.append(nc.partition_id_tensor.name)

    def _body(*args):
        operands = list(args)
        if nc.partition_id_tensor is not None:
            operands.append(bass2jax.partition_id_tensor())
        outs = bass2jax._bass_exec_p.bind(
            *operands,
            out_avals=tuple(out_avals),
            in_names=tuple(all_names),
            out_names=tuple(out_names),
            lowering_input_output_aliases=(),
            sim_require_finite=True,
            sim_require_nnan=True,
            nc=nc,
        )
        return tuple(outs)

    devices = jax.devices()[:N_CORES]
    mesh = Mesh(np.asarray(devices), ("core",))
    n_outs = len(out_names)
    sharded = jax.jit(
        shard_map(
            _body,
            mesh=mesh,
            in_specs=(PartitionSpec("core"),) * (n_params + n_outs),
            out_specs=(PartitionSpec("core"),) * n_outs,
            check_rep=False,
        ),
        keep_unused=True,
    )

    def runner(qt_all, kt_all, vv_all):
        zeros = [
            np.zeros((N_CORES * s[0], *s[1:]), dt) for (s, dt) in zero_shapes
        ]
        (out_all,) = sharded(qt_all, kt_all, vv_all, *zeros)
        return np.asarray(out_all)

    runner.sharded = sharded
    runner.mesh = mesh
    runner.zero_shapes = zero_shapes
    return runner


_RUNNERS = {}


def _pack_inputs(unit_specs, q, k, v, n_ctx):
    """unit_specs: list of (qk_batch, head, v_batch0, v_batch1)."""
    import ml_dtypes

    T = n_ctx // 128
    NU = len(unit_specs)
    qk_np = np.float16 if QK_F16 else np.float32
    q_pre = np.float32(2.0**-4 if QK_F16 else 2.0**-8)
    k_pre = np.float32(2.0**-4 if QK_F16 else 1.0)
    qt_all = np.empty((NU, 128, T * 128), qk_np)
    kt_all = np.empty((NU, 128, (T // 2) * 128), qk_np)
    vv_all = np.empty((NU, 128, T * 129), ml_dtypes.bfloat16)
    vtmp = np.empty((T, 128, 129), np.float32)
    vtmp[:, :, 128] = 1.0
    for i, (bq, h, b0, b1) in enumerate(unit_specs):
        qT = q[bq, h].T * q_pre              # [64, n_ctx], d-major, prescaled
        qt_all[i, 0:64] = qT
        qt_all[i, 64:128] = qT               # replica feeds PE row-group 1
        kT = (k[bq, h].T * k_pre).reshape(64, T, 128)  # [dd, t, r]
        kt_all[i, 0:64] = kT[:, 0::2, :].reshape(64, -1)
        kt_all[i, 64:128] = kT[:, 1::2, :].reshape(64, -1)
        vtmp[:, :, 0:64] = v[b0, h].reshape(T, 128, D)
        vtmp[:, :, 64:128] = v[b1, h].reshape(T, 128, D)
        vv_all[i] = (
            vtmp.transpose(1, 0, 2).reshape(128, -1).astype(ml_dtypes.bfloat16)
        )
    return qt_all, kt_all, vv_all


def _run_units(unit_specs, q, k, v, n_ctx):
    n_units = len(unit_specs) // N_CORES
    assert n_units * N_CORES == len(unit_specs)
    key = (n_units, n_ctx)
    if key not in _RUNNERS:
        _RUNNERS[key] = _get_runner(n_units, n_ctx)
    runner = _RUNNERS[key]

    qt_all, kt_all, vv_all = _pack_inputs(unit_specs, q, k, v, n_ctx)
    out_all = runner(qt_all, kt_all, vv_all)  # [NU, n_ctx, 128]

    out = np.empty((B, H, n_ctx, D), np.float32)
    for i, (bq, h, b0, b1) in enumerate(unit_specs):
        out[b0, h] = out_all[i, :, 0:64]
        if b1 != b0:
            out[b1, h] = out_all[i, :, 64:128]
    return out


def kernel(q, k, v, swap):
    q = np.ascontiguousarray(np.asarray(q, dtype=np.float32))
    k = np.ascontiguousarray(np.asarray(k, dtype=np.float32))
    v = np.ascontiguousarray(np.asarray(v, dtype=np.float32))
    swap_val = int(np.asarray(swap).reshape(-1)[0])

    n_ctx = q.shape[2]
    if swap_val:
        # 32 pair-units: attn of (b, h) applied to v[b] and v[b + B//2]
        specs = [(bq, h, bq, bq + B // 2) for bq in range(B // 2) for h in range(H)]
    else:
        # 64 independent units (2nd v slot duplicates the 1st)
        specs = [(b, h, b, b) for b in range(B) for h in range(H)]
    return _run_units(specs, q, k, v, n_ctx)


if __name__ == "__main__":
    rng = np.random.default_rng(0)
    q = rng.standard_normal((B, H, N, D), dtype=np.float32)
    k = rng.standard_normal((B, H, N, D), dtype=np.float32)
    v = rng.standard_normal((B, H, N, D), dtype=np.float32)
    o = kernel(q, k, v, 1)
    print("out", o.shape, o.dtype, float(np.abs(o).mean()))
